# revision 11
# baseline (speedup 1.0000x reference)
"""AttackNet kernel for 8 Trainium2 NeuronCores (v2: bf16 + engine split).

Reference computation:
    out  = conv1x1(x, W) + b                        # 60 channels
    pert = out.reshape(n, 20, 3, h, w)[arange, target]
    pert = ((pert - min) / (max - min) - 0.5) * 2   # per (sample, channel) spatial
    return pert * (MAX_PERTURBATION / 128)

Only the 3 gathered channels per sample are needed; the host picks the
per-sample 3x3 weight block.  The bias cancels inside the min/max
normalization and is dropped.  The normalization is invariant to a
per-(sample,j) rescale of lin, so weights are divided by a pivot w_p
(sign handled in the final affine):
    q_j   = x_a * (w_a/w_p) + x_p          (scalar_tensor_tensor)
    lin_j = x_b * (w_b/w_p) + q_j          (custom DVE op, fused running
                                            max -> pad col, min -> accum)
    out_j = lin_j * s + t,  s = 2*sign/rng, t = -min*s - sign

Sharding: pure data parallel, 4 samples per core across 8 cores.
Data is bf16 end-to-end (host casts both ways); rel-err budget 2e-2.

Engine split per core (free dim 1568, 128 partitions = 4 samples x 32):
    Pool (gpsimd): q pad memsets, q0's add (tensor_tensor)
    DVE:  per-j tmp = x_a*ra (tensor_scalar 4x), q1/q2 adds (tensor_tensor
          2x), LINSTAT x3 (1x, fused running-max + min accum), per-j
          stats arithmetic, final j0 (tensor_scalar 2x)
    PE:   stat transposes + per-sample -> partition broadcast matmul
    ACT:  finals j1/j2 (activation Identity), PSUM->SBUF stat copies
"""

import sys
import time

sys.path.insert(0, "/opt/trn_rl_repo")
sys.path.insert(0, "/root/problem")

import numpy as np

import concourse.bass as bass  # noqa: F401
import concourse.tile as tile
from concourse import bacc, bass_isa, mybir
from concourse.bass_utils import run_bass_kernel_spmd

try:
    from ml_dtypes import bfloat16 as np_bf16
except ImportError:
    import jax.numpy as _jnp

    np_bf16 = _jnp.bfloat16


def _install_ntff_hook_shim():
    """Provide antenv.axon_hooks (absent in this image) so trace=True works."""
    import types

    if "antenv.axon_hooks" in sys.modules:
        return
    import contextlib
    import ctypes

    so_path = "/opt/axon/libaxon_pjrt.so"
    try:
        lib = ctypes.CDLL(so_path)
        lib.axon_start_nrt_profile.argtypes = [
            ctypes.POINTER(ctypes.c_int64),
            ctypes.c_size_t,
        ]
        lib.axon_start_nrt_profile.restype = ctypes.c_int64
        lib.axon_stop_nrt_profile.argtypes = [ctypes.c_char_p]
        lib.axon_stop_nrt_profile.restype = ctypes.c_int64
    except OSError:
        lib = None

    @contextlib.contextmanager
    def _hook(output_dir, device_ids):
        import jax

        jax.devices()
        if device_ids:
            ids = (ctypes.c_int64 * len(device_ids))(*device_ids)
            rc = lib.axon_start_nrt_profile(ids, len(device_ids))
        else:
            rc = lib.axon_start_nrt_profile(None, 0)
        if rc != 0:
            raise RuntimeError(f"axon_start_nrt_profile rc={rc}")
        try:
            yield
        finally:
            n = lib.axon_stop_nrt_profile(str(output_dir).encode())
            print(f"ntff profile: {n} file(s) written to {output_dir}",
                  file=sys.stderr)

    mod = types.ModuleType("antenv.axon_hooks")
    mod.get_axon_ntff_profile_hook = lambda: (_hook if lib is not None else None)
    mod.set_axon_ntff_profile_hook = lambda h: None
    import antenv

    antenv.axon_hooks = mod
    sys.modules["antenv.axon_hooks"] = mod


_install_ntff_hook_shim()

# registers the custom DVE op in concourse.dve_ops at import time
from concourse import dve_ops
from concourse.dve_spec import (
    AluOp, C0, C1, C2, Spec, Src0, Src1, lower, scan, select,
)
from concourse.dve_uop import DveOpSpec


def _linstat_ref(in0, in1, c0, c1, c2):
    a0 = np.asarray(in0, np.float32)
    a1 = np.asarray(in1, np.float32)
    v = (a0 * c0 + a1).astype(np.float32)
    r = np.maximum.accumulate(v, axis=-1)
    o = np.where(a1 <= c1, r, v)
    acc = np.minimum(
        np.float32(c2), o.reshape(o.shape[0], -1).min(-1, keepdims=True)
    )
    return o, acc


def _register(name, spec):
    for op in dve_ops.OPS:
        if op.name == name:
            return op
    opcode = dve_ops._CUSTOM_DVE_ROW_BASE + len(dve_ops.OPS)
    assert opcode < 0x20
    shas = {}
    for ver in ("v3", "v4"):
        uops = lower(spec, ver=ver)
        shas[ver] = DveOpSpec(
            name=name, opcode=opcode, uops=uops, rd1_en=True
        ).sha(ver)
    op = dve_ops.DveOp(name, spec, subdim=False, uops_sha=shas)
    dve_ops.OPS.append(op)
    dve_ops.CUSTOM_DVE_SPECS[name] = spec
    dve_ops._SUB_OPCODE_FOR_NAME[name] = opcode
    return op


_v = Src0 * C0 + Src1
LINSTAT = _register(
    "LINSTAT_ATK",
    Spec(
        body=select(Src1 <= C1, scan(AluOp.MAX, _v), _v),
        accum=AluOp.MIN,
        accum_init=C2,
        reference=_linstat_ref,
    ),
)

P = 128                 # SBUF partitions
H = W_ = 224
F = H * W_              # 50176 spatial elements per plane
G32 = 32                # partitions per sample group
NS = 4                  # samples per core
FD = F // G32           # 1568 free elements per partition
NCORES = 8
N = NCORES * NS         # 32 samples total
OUT_CHANNELS = 3
MAX_PERTURBATION = 128.0
OUT_SCALE = MAX_PERTURBATION / 128.0  # == 1.0
PAD_SENTINEL = -3.0e38

_CACHE = {}


def _build(roles):
    """roles[j] = (a, p, b): q_j = x_a*ra + x_p; lin_j = x_b*rb + q_j."""
    f32 = mybir.dt.float32
    bf16 = mybir.dt.bfloat16
    mult = mybir.AluOpType.mult
    add = mybir.AluOpType.add
    div = mybir.AluOpType.divide

    nc = bacc.Bacc(
        "TRN2", target_bir_lowering=False, debug=False, num_devices=1
    )
    # host-padded, partition-major: xs[c, p, :] with p = sample*32 + block
    xs = nc.dram_tensor("xs", [3, P, FD + 1], bf16, kind="ExternalInput")
    # f32 aux, one DMA: cols 0-15 smalls (ra/rb cols 0-5 per partition,
    # rows 0-3 cols 6-8 = 2*sigma_j, cols 9-11 = -sigma_j, rows 0-1 cols
    # 12-13 = 2x2 identity), cols 16-143 = 128x128 identity, rows 0-3
    # cols 144-271 = gmat (sample -> partition broadcast weights)
    aux = nc.dram_tensor("aux", [P, 272], f32, kind="ExternalInput")
    out = nc.dram_tensor("out", [3, P, FD], bf16, kind="ExternalOutput")

    with tile.TileContext(nc) as tc:
        with (
            tc.tile_pool(name="wp", bufs=1) as wp,
            tc.tile_pool(name="xp", bufs=1) as xp,
            tc.tile_pool(name="qp", bufs=1) as qp,
            tc.tile_pool(name="lp", bufs=1) as lp,
            tc.tile_pool(name="st", bufs=1) as st,
            tc.tile_pool(name="pp", bufs=2, space="PSUM") as pp,
            tc.tile_pool(name="op", bufs=1) as outp,
        ):
            xts = [
                xp.tile([P, FD + 1], bf16, tag=f"x{c}", name=f"x{c}")
                for c in range(3)
            ]
            aux_t = wp.tile([P, 272], f32)

            # Load order: the first q computed is j=1 (tmp1 = x_a1*ra then
            # q1 = tmp1 + x_p1), so x_a1 gets ring A to itself while ring B
            # carries smf + x_p1; the remaining channel follows.
            a1, p1 = roles[1][0], roles[1][1]
            third = [c for c in range(3) if c not in (a1, p1)][0]
            nc.scalar.dma_start(aux_t[:], aux[:])
            nc.sync.dma_start(xts[a1][:], xs[a1])
            nc.scalar.dma_start(xts[p1][:], xs[p1])
            nc.sync.dma_start(xts[third][:], xs[third])
            smf_t = aux_t[:, 0:16]
            identf_t = aux_t[:, 16 : 16 + P]
            g_t = aux_t[:NS, 16 + P : 16 + 2 * P]
            ident2 = aux_t[:2, 12:14]

            ra = lambda j: aux_t[:, 2 * j : 2 * j + 1]       # noqa: E731
            rb = lambda j: aux_t[:, 2 * j + 1 : 2 * j + 2]   # noqa: E731
            sg2 = lambda j: aux_t[:NS, 6 + j : 7 + j]        # noqa: E731
            nsg = lambda j: aux_t[:NS, 9 + j : 10 + j]       # noqa: E731

            # q tiles with sentinel pad col (memset on Pool, cheap+early)
            qts = []
            for j in range(3):
                q = qp.tile([P, FD + 1], bf16, tag=f"q{j}", name=f"q{j}")
                nc.gpsimd.memset(q[:, FD : FD + 1], PAD_SENTINEL)
                qts.append(q)

            lins = {}
            stfs = {}

            def q_tmp(j):
                a, p, _ = roles[j]
                tmp = qp.tile([P, FD], bf16, tag=f"tmp{j}", name=f"tmp{j}")
                nc.vector.tensor_scalar(
                    tmp[:], xts[a][:, 0:FD], ra(j), None, op0=mult
                )
                return tmp

            def q_add_dve(j, tmp):
                _, p, _ = roles[j]
                nc.vector.tensor_tensor(
                    qts[j][:, 0:FD], tmp[:], xts[p][:, 0:FD], op=add
                )

            def linstat(j):
                _, _, b = roles[j]
                lin = lp.tile([P, FD + 2], f32, tag=f"lin{j}", name=f"lin{j}")
                nc.vector._custom_dve(
                    LINSTAT, out=lin[:, 0 : FD + 1],
                    in0=xts[b][:, 0 : FD + 1], in1=qts[j][:, 0 : FD + 1],
                    s0=rb(j), s1=-1.0e38, imm2=3.4e38,
                    accum_out=lin[:, FD + 1 : FD + 2],
                )
                lins[j] = lin

            def chain_neg(j):
                # negate min so one grouped max-reduce covers both cols
                lin = lins[j]
                nc.vector.tensor_scalar_mul(
                    lin[:, FD + 1 : FD + 2], lin[:, FD + 1 : FD + 2], -1.0
                )

            def chain_t1(j):
                ps1 = pp.tile([2, P], f32, tag="ps1", name=f"ps1_{j}")
                nc.tensor.transpose(
                    ps1[:], lins[j][:, FD : FD + 2], identf_t
                )
                return ps1

            def chain_red(j, ps1):
                # [2,128] -> per-sample [2,4] -> [4,2] -> s,t -> [128,2]
                st4 = st.tile([2, NS], f32, tag=f"st4_{j}", name=f"st4_{j}")
                nc.vector.tensor_reduce(
                    st4[:], ps1[:].rearrange("r (n g) -> r n g", g=G32),
                    axis=mybir.AxisListType.X, op=mybir.AluOpType.max,
                )
                ps2 = pp.tile([NS, 2], f32, tag="ps2", name=f"ps2_{j}")
                nc.tensor.transpose(ps2[:], st4[:], ident2)
                r2 = st.tile([NS, 2], f32, tag=f"r2_{j}", name=f"r2_{j}")
                nc.vector.tensor_copy(r2[:], ps2[:])
                st2 = st.tile([NS, 2], f32, tag=f"st2_{j}", name=f"st2_{j}")
                rr = st.tile([NS, 1], f32, tag=f"rr_{j}", name=f"rr_{j}")
                rcp = st.tile([NS, 1], f32, tag=f"rcp_{j}", name=f"rcp_{j}")
                nc.vector.tensor_tensor(rr[:], r2[:, 0:1], r2[:, 1:2], op=add)
                nc.vector.reciprocal(rcp[:], rr[:])
                nc.vector.tensor_scalar(
                    st2[:, 0:1], rcp[:], sg2(j), None, op0=mult
                )
                nc.vector.tensor_scalar(
                    st2[:, 1:2], r2[:, 1:2], st2[:, 0:1], nsg(j),
                    op0=mult, op1=add,
                )
                ps3 = pp.tile([P, 2], f32, tag="ps3", name=f"ps3_{j}")
                nc.tensor.matmul(ps3[:], g_t, st2[:], start=True, stop=True)
                stf = st.tile([P, 2], f32, tag=f"stf{j}", name=f"stf{j}")
                nc.scalar.copy(stf[:], ps3[:])
                stfs[j] = stf

            def final_act(j, ring):
                ot = outp.tile([P, FD], bf16, tag=f"ot{j}", name=f"ot{j}")
                nc.scalar.activation(
                    ot[:], lins[j][:, 0:FD],
                    mybir.ActivationFunctionType.Identity,
                    bias=stfs[j][:, 1:2], scale=stfs[j][:, 0:1],
                )
                ring.dma_start(out[j], ot[:])

            def final_dve(j, ring):
                ot = outp.tile([P, FD], bf16, tag=f"ot{j}", name=f"ot{j}")
                nc.vector.tensor_scalar(
                    ot[:], lins[j][:, 0:FD], stfs[j][:, 0:1],
                    stfs[j][:, 1:2], op0=mult, op1=add,
                )
                ring.dma_start(out[j], ot[:])

            # DVE queue: q pairs (1,2,0), LINSTATs (1,2,0) with each
            # chain's small ops interleaved one LINSTAT later so the DVE
            # never stalls on the PE transpose it just seeded.  ACT queue:
            # stat copies first, then the one big ACT final (j1); j2/j0
            # finals run on the DVE in the chain gaps.
            tt1 = q_tmp(1)
            q_add_dve(1, tt1)
            tt2 = q_tmp(2)
            q_add_dve(2, tt2)
            tt0 = q_tmp(0)
            q_add_dve(0, tt0)
            linstat(1)
            chain_neg(1)
            ps_1 = chain_t1(1)
            linstat(2)
            chain_red(1, ps_1)
            chain_neg(2)
            ps_2 = chain_t1(2)
            linstat(0)
            chain_red(2, ps_2)
            chain_neg(0)
            ps_0 = chain_t1(0)
            chain_red(0, ps_0)
            final_act(1, nc.sync)
            final_act(2, nc.sync)
            final_dve(0, nc.sync)

    nc.compile()
    return nc


def get_nc(roles):
    key = tuple(roles)
    if key not in _CACHE:
        _CACHE[key] = _build(key)
    return _CACHE[key]


def _choose_roles(Wsel):
    """Wsel: (N, 3 out, 3 in). Per j pick pivot maximizing min_s |w_p|."""
    roles = []
    for j in range(3):
        w = np.abs(Wsel[:, j, :])  # (N, 3)
        p = int(np.argmax(w.min(axis=0)))
        a, b = [c for c in range(3) if c != p]
        roles.append((a, p, b))
    return tuple(roles)


def make_in_maps(x, target, W, b):
    x = np.ascontiguousarray(np.asarray(x), dtype=np.float32)
    tgt = np.asarray(target).astype(np.int64)
    Wm = np.asarray(W, dtype=np.float32).reshape(20 * OUT_CHANNELS, 3)
    Wsel = Wm.reshape(20, OUT_CHANNELS, 3)[tgt]  # (N, 3 out, 3 in)
    roles = _choose_roles(Wsel)

    xpad = np.zeros((N, 3, G32, FD + 1), dtype=np_bf16)
    xpad[:, :, :, :FD] = x.reshape(N, 3, G32, FD).astype(np_bf16)

    in_maps = []
    for core in range(NCORES):
        lo = core * NS
        xsc = np.ascontiguousarray(
            xpad[lo : lo + NS].transpose(1, 0, 2, 3).reshape(3, P, FD + 1)
        )
        aux = np.zeros((P, 272), dtype=np.float32)
        for j, (a, p, bb) in enumerate(roles):
            wp = Wsel[lo : lo + NS, j, p]          # (NS,)
            rra = Wsel[lo : lo + NS, j, a] / wp
            rrb = Wsel[lo : lo + NS, j, bb] / wp
            sg = np.sign(wp).astype(np.float32)
            aux[:, 2 * j] = np.repeat(rra, G32)
            aux[:, 2 * j + 1] = np.repeat(rrb, G32)
            aux[:NS, 6 + j] = 2.0 * OUT_SCALE * sg
            aux[:NS, 9 + j] = -OUT_SCALE * sg
        aux[0, 12] = 1.0
        aux[1, 13] = 1.0
        aux[:, 16 : 16 + P] = np.eye(P, dtype=np.float32)
        aux[:NS, 16 + P : 16 + 2 * P] = np.repeat(
            np.eye(NS, dtype=np.float32), G32, axis=1
        )
        in_maps.append({"xs": xsc, "aux": aux})
    return in_maps, roles


def run(x, target, W, b, trace=False, retries=2):
    in_maps, roles = make_in_maps(x, target, W, b)
    nc = get_nc(roles)
    last_err = None
    for attempt in range(retries + 1):
        try:
            res = run_bass_kernel_spmd(
                nc, in_maps, list(range(NCORES)), trace=trace
            )
            outs = []
            for r in res.results:
                o = np.asarray(r["out"]).astype(np.float32)
                o = o.reshape(3, NS, G32, FD).transpose(1, 0, 2, 3)
                outs.append(o.reshape(NS, OUT_CHANNELS, H, W_))
            return np.concatenate(outs, axis=0), res
        except Exception as e:  # device may need recovery; retry
            last_err = e
            if attempt < retries:
                time.sleep(20)
    raise last_err


def kernel(x, target, W, b):
    out, _ = run(x, target, W, b)
    return out


# revision 12
# speedup vs baseline: 1.0076x; 1.0076x over previous
"""AttackNet kernel for 8 Trainium2 NeuronCores (v2: bf16 + engine split).

Reference computation:
    out  = conv1x1(x, W) + b                        # 60 channels
    pert = out.reshape(n, 20, 3, h, w)[arange, target]
    pert = ((pert - min) / (max - min) - 0.5) * 2   # per (sample, channel) spatial
    return pert * (MAX_PERTURBATION / 128)

Only the 3 gathered channels per sample are needed; the host picks the
per-sample 3x3 weight block.  The bias cancels inside the min/max
normalization and is dropped.  The normalization is invariant to a
per-(sample,j) rescale of lin, so weights are divided by a pivot w_p
(sign handled in the final affine):
    q_j   = x_a * (w_a/w_p) + x_p          (scalar_tensor_tensor)
    lin_j = x_b * (w_b/w_p) + q_j          (custom DVE op, fused running
                                            max -> pad col, min -> accum)
    out_j = lin_j * s + t,  s = 2*sign/rng, t = -min*s - sign

Sharding: pure data parallel, 4 samples per core across 8 cores.
Data is bf16 end-to-end (host casts both ways); rel-err budget 2e-2.

Engine split per core (free dim 1568, 128 partitions = 4 samples x 32):
    Pool (gpsimd): q pad memsets, q0's add (tensor_tensor)
    DVE:  per-j tmp = x_a*ra (tensor_scalar 4x), q1/q2 adds (tensor_tensor
          2x), LINSTAT x3 (1x, fused running-max + min accum), per-j
          stats arithmetic, final j0 (tensor_scalar 2x)
    PE:   stat transposes + per-sample -> partition broadcast matmul
    ACT:  finals j1/j2 (activation Identity), PSUM->SBUF stat copies
"""

import sys
import time

sys.path.insert(0, "/opt/trn_rl_repo")
sys.path.insert(0, "/root/problem")

import numpy as np

import concourse.bass as bass  # noqa: F401
import concourse.tile as tile
from concourse import bacc, bass_isa, mybir
from concourse.bass_utils import run_bass_kernel_spmd

try:
    from ml_dtypes import bfloat16 as np_bf16
except ImportError:
    import jax.numpy as _jnp

    np_bf16 = _jnp.bfloat16


def _install_ntff_hook_shim():
    """Provide antenv.axon_hooks (absent in this image) so trace=True works."""
    import types

    if "antenv.axon_hooks" in sys.modules:
        return
    import contextlib
    import ctypes

    so_path = "/opt/axon/libaxon_pjrt.so"
    try:
        lib = ctypes.CDLL(so_path)
        lib.axon_start_nrt_profile.argtypes = [
            ctypes.POINTER(ctypes.c_int64),
            ctypes.c_size_t,
        ]
        lib.axon_start_nrt_profile.restype = ctypes.c_int64
        lib.axon_stop_nrt_profile.argtypes = [ctypes.c_char_p]
        lib.axon_stop_nrt_profile.restype = ctypes.c_int64
    except OSError:
        lib = None

    @contextlib.contextmanager
    def _hook(output_dir, device_ids):
        import jax

        jax.devices()
        if device_ids:
            ids = (ctypes.c_int64 * len(device_ids))(*device_ids)
            rc = lib.axon_start_nrt_profile(ids, len(device_ids))
        else:
            rc = lib.axon_start_nrt_profile(None, 0)
        if rc != 0:
            raise RuntimeError(f"axon_start_nrt_profile rc={rc}")
        try:
            yield
        finally:
            n = lib.axon_stop_nrt_profile(str(output_dir).encode())
            print(f"ntff profile: {n} file(s) written to {output_dir}",
                  file=sys.stderr)

    mod = types.ModuleType("antenv.axon_hooks")
    mod.get_axon_ntff_profile_hook = lambda: (_hook if lib is not None else None)
    mod.set_axon_ntff_profile_hook = lambda h: None
    import antenv

    antenv.axon_hooks = mod
    sys.modules["antenv.axon_hooks"] = mod


_install_ntff_hook_shim()

# registers the custom DVE op in concourse.dve_ops at import time
from concourse import dve_ops
from concourse.dve_spec import (
    AluOp, C0, C1, C2, Spec, Src0, Src1, lower, scan, select,
)
from concourse.dve_uop import DveOpSpec


def _linstat_ref(in0, in1, c0, c1, c2):
    a0 = np.asarray(in0, np.float32)
    a1 = np.asarray(in1, np.float32)
    v = (a0 * c0 + a1).astype(np.float32)
    r = np.maximum.accumulate(v, axis=-1)
    o = np.where(a1 <= c1, r, v)
    acc = np.minimum(
        np.float32(c2), o.reshape(o.shape[0], -1).min(-1, keepdims=True)
    )
    return o, acc


def _register(name, spec):
    for op in dve_ops.OPS:
        if op.name == name:
            return op
    opcode = dve_ops._CUSTOM_DVE_ROW_BASE + len(dve_ops.OPS)
    assert opcode < 0x20
    shas = {}
    for ver in ("v3", "v4"):
        uops = lower(spec, ver=ver)
        shas[ver] = DveOpSpec(
            name=name, opcode=opcode, uops=uops, rd1_en=True
        ).sha(ver)
    op = dve_ops.DveOp(name, spec, subdim=False, uops_sha=shas)
    dve_ops.OPS.append(op)
    dve_ops.CUSTOM_DVE_SPECS[name] = spec
    dve_ops._SUB_OPCODE_FOR_NAME[name] = opcode
    return op


_v = Src0 * C0 + Src1
LINSTAT = _register(
    "LINSTAT_ATK",
    Spec(
        body=select(Src1 <= C1, scan(AluOp.MAX, _v), _v),
        accum=AluOp.MIN,
        accum_init=C2,
        reference=_linstat_ref,
    ),
)

P = 128                 # SBUF partitions
H = W_ = 224
F = H * W_              # 50176 spatial elements per plane
G32 = 32                # partitions per sample group
NS = 4                  # samples per core
FD = F // G32           # 1568 free elements per partition
NCORES = 8
N = NCORES * NS         # 32 samples total
OUT_CHANNELS = 3
MAX_PERTURBATION = 128.0
OUT_SCALE = MAX_PERTURBATION / 128.0  # == 1.0
PAD_SENTINEL = -3.0e38

_CACHE = {}


def _build(roles):
    """roles[j] = (a, p, b): q_j = x_a*ra + x_p; lin_j = x_b*rb + q_j."""
    f32 = mybir.dt.float32
    bf16 = mybir.dt.bfloat16
    mult = mybir.AluOpType.mult
    add = mybir.AluOpType.add
    div = mybir.AluOpType.divide

    nc = bacc.Bacc(
        "TRN2", target_bir_lowering=False, debug=False, num_devices=1
    )
    # host-padded, partition-major: xs[c, p, :] with p = sample*32 + block
    xs = nc.dram_tensor("xs", [3, P, FD + 1], bf16, kind="ExternalInput")
    # f32 aux, one DMA: cols 0-15 smalls (ra/rb cols 0-5 per partition,
    # rows 0-3 cols 6-8 = 2*sigma_j, cols 9-11 = -sigma_j, rows 0-1 cols
    # 12-13 = 2x2 identity), cols 16-143 = 128x128 identity, rows 0-3
    # cols 144-271 = gmat (sample -> partition broadcast weights)
    aux = nc.dram_tensor("aux", [P, 272], f32, kind="ExternalInput")
    out = nc.dram_tensor("out", [3, P, FD], bf16, kind="ExternalOutput")

    with tile.TileContext(nc) as tc:
        with (
            tc.tile_pool(name="wp", bufs=1) as wp,
            tc.tile_pool(name="xp", bufs=1) as xp,
            tc.tile_pool(name="qp", bufs=1) as qp,
            tc.tile_pool(name="lp", bufs=1) as lp,
            tc.tile_pool(name="st", bufs=1) as st,
            tc.tile_pool(name="pp", bufs=2, space="PSUM") as pp,
            tc.tile_pool(name="op", bufs=1) as outp,
        ):
            xts = [
                xp.tile([P, FD + 1], bf16, tag=f"x{c}", name=f"x{c}")
                for c in range(3)
            ]
            aux_t = wp.tile([P, 272], f32)

            # Load order: the first q computed is j=1 (tmp1 = x_a1*ra then
            # q1 = tmp1 + x_p1), so x_a1 gets ring A to itself while ring B
            # carries smf + x_p1; the remaining channel follows.
            a1, p1 = roles[1][0], roles[1][1]
            third = [c for c in range(3) if c not in (a1, p1)][0]
            HF = 784
            nc.scalar.dma_start(aux_t[:], aux[:])
            nc.sync.dma_start(xts[a1][:, 0:HF], xs[a1][:, 0:HF])
            nc.scalar.dma_start(xts[p1][:, 0:HF], xs[p1][:, 0:HF])
            nc.sync.dma_start(xts[a1][:, HF:], xs[a1][:, HF:])
            nc.scalar.dma_start(xts[p1][:, HF:], xs[p1][:, HF:])
            nc.sync.dma_start(xts[third][:, 0:HF], xs[third][:, 0:HF])
            nc.scalar.dma_start(xts[third][:, HF:], xs[third][:, HF:])
            smf_t = aux_t[:, 0:16]
            identf_t = aux_t[:, 16 : 16 + P]
            g_t = aux_t[:NS, 16 + P : 16 + 2 * P]
            ident2 = aux_t[:2, 12:14]

            ra = lambda j: aux_t[:, 2 * j : 2 * j + 1]       # noqa: E731
            rb = lambda j: aux_t[:, 2 * j + 1 : 2 * j + 2]   # noqa: E731
            sg2 = lambda j: aux_t[:NS, 6 + j : 7 + j]        # noqa: E731
            nsg = lambda j: aux_t[:NS, 9 + j : 10 + j]       # noqa: E731

            # q tiles with sentinel pad col (memset on Pool, cheap+early)
            qts = []
            for j in range(3):
                q = qp.tile([P, FD + 1], bf16, tag=f"q{j}", name=f"q{j}")
                nc.gpsimd.memset(q[:, FD : FD + 1], PAD_SENTINEL)
                qts.append(q)

            lins = {}
            stfs = {}

            def q_tmp(j, h):
                a, p, _ = roles[j]
                if h == 0:
                    tmp = qp.tile(
                        [P, FD], bf16, tag=f"tmp{j}", name=f"tmp{j}"
                    )
                else:
                    tmp = tmps[j]
                lo, hi = (0, 784) if h == 0 else (784, FD)
                nc.vector.tensor_scalar(
                    tmp[:, lo:hi], xts[a][:, lo:hi], ra(j), None, op0=mult
                )
                return tmp

            def q_add_dve(j, h):
                _, p, _ = roles[j]
                lo, hi = (0, 784) if h == 0 else (784, FD)
                nc.vector.tensor_tensor(
                    qts[j][:, lo:hi], tmps[j][:, lo:hi],
                    xts[p][:, lo:hi], op=add
                )

            def linstat(j):
                _, _, b = roles[j]
                lin = lp.tile([P, FD + 2], f32, tag=f"lin{j}", name=f"lin{j}")
                nc.vector._custom_dve(
                    LINSTAT, out=lin[:, 0 : FD + 1],
                    in0=xts[b][:, 0 : FD + 1], in1=qts[j][:, 0 : FD + 1],
                    s0=rb(j), s1=-1.0e38, imm2=3.4e38,
                    accum_out=lin[:, FD + 1 : FD + 2],
                )
                lins[j] = lin

            def chain_neg(j):
                # negate min so one grouped max-reduce covers both cols
                lin = lins[j]
                nc.vector.tensor_scalar_mul(
                    lin[:, FD + 1 : FD + 2], lin[:, FD + 1 : FD + 2], -1.0
                )

            def chain_t1(j):
                ps1 = pp.tile([2, P], f32, tag="ps1", name=f"ps1_{j}")
                nc.tensor.transpose(
                    ps1[:], lins[j][:, FD : FD + 2], identf_t
                )
                return ps1

            def chain_red(j, ps1):
                # [2,128] -> per-sample [2,4] -> [4,2] -> s,t -> [128,2]
                st4 = st.tile([2, NS], f32, tag=f"st4_{j}", name=f"st4_{j}")
                nc.vector.tensor_reduce(
                    st4[:], ps1[:].rearrange("r (n g) -> r n g", g=G32),
                    axis=mybir.AxisListType.X, op=mybir.AluOpType.max,
                )
                ps2 = pp.tile([NS, 2], f32, tag="ps2", name=f"ps2_{j}")
                nc.tensor.transpose(ps2[:], st4[:], ident2)
                r2 = st.tile([NS, 2], f32, tag=f"r2_{j}", name=f"r2_{j}")
                nc.vector.tensor_copy(r2[:], ps2[:])
                st2 = st.tile([NS, 2], f32, tag=f"st2_{j}", name=f"st2_{j}")
                rr = st.tile([NS, 1], f32, tag=f"rr_{j}", name=f"rr_{j}")
                rcp = st.tile([NS, 1], f32, tag=f"rcp_{j}", name=f"rcp_{j}")
                nc.vector.tensor_tensor(rr[:], r2[:, 0:1], r2[:, 1:2], op=add)
                nc.vector.reciprocal(rcp[:], rr[:])
                nc.vector.tensor_scalar(
                    st2[:, 0:1], rcp[:], sg2(j), None, op0=mult
                )
                nc.vector.tensor_scalar(
                    st2[:, 1:2], r2[:, 1:2], st2[:, 0:1], nsg(j),
                    op0=mult, op1=add,
                )
                ps3 = pp.tile([P, 2], f32, tag="ps3", name=f"ps3_{j}")
                nc.tensor.matmul(ps3[:], g_t, st2[:], start=True, stop=True)
                stf = st.tile([P, 2], f32, tag=f"stf{j}", name=f"stf{j}")
                nc.scalar.copy(stf[:], ps3[:])
                stfs[j] = stf

            def final_act(j, ring):
                ot = outp.tile([P, FD], bf16, tag=f"ot{j}", name=f"ot{j}")
                nc.scalar.activation(
                    ot[:], lins[j][:, 0:FD],
                    mybir.ActivationFunctionType.Identity,
                    bias=stfs[j][:, 1:2], scale=stfs[j][:, 0:1],
                )
                ring.dma_start(out[j], ot[:])

            def final_dve(j, ring):
                ot = outp.tile([P, FD], bf16, tag=f"ot{j}", name=f"ot{j}")
                nc.vector.tensor_scalar(
                    ot[:], lins[j][:, 0:FD], stfs[j][:, 0:1],
                    stfs[j][:, 1:2], op0=mult, op1=add,
                )
                ring.dma_start(out[j], ot[:])

            # DVE queue: q pairs (1,2,0), LINSTATs (1,2,0) with each
            # chain's small ops interleaved one LINSTAT later so the DVE
            # never stalls on the PE transpose it just seeded.  ACT queue:
            # stat copies first, then the one big ACT final (j1); j2/j0
            # finals run on the DVE in the chain gaps.
            tmps = {}
            tmps[1] = q_tmp(1, 0)
            q_add_dve(1, 0)
            tmps[2] = q_tmp(2, 0)
            q_tmp(1, 1)
            q_add_dve(1, 1)
            q_add_dve(2, 0)
            tmps[0] = q_tmp(0, 0)
            q_tmp(2, 1)
            q_add_dve(2, 1)
            q_tmp(0, 1)
            q_add_dve(0, 0)
            q_add_dve(0, 1)
            linstat(1)
            chain_neg(1)
            ps_1 = chain_t1(1)
            linstat(2)
            chain_red(1, ps_1)
            chain_neg(2)
            ps_2 = chain_t1(2)
            linstat(0)
            chain_red(2, ps_2)
            chain_neg(0)
            ps_0 = chain_t1(0)
            chain_red(0, ps_0)
            final_act(1, nc.sync)
            final_dve(0, nc.sync)
            final_dve(2, nc.sync)

    nc.compile()
    return nc


def get_nc(roles):
    key = tuple(roles)
    if key not in _CACHE:
        _CACHE[key] = _build(key)
    return _CACHE[key]


def _choose_roles(Wsel):
    """Wsel: (N, 3 out, 3 in). Per j pick pivot maximizing min_s |w_p|."""
    roles = []
    for j in range(3):
        w = np.abs(Wsel[:, j, :])  # (N, 3)
        p = int(np.argmax(w.min(axis=0)))
        a, b = [c for c in range(3) if c != p]
        roles.append((a, p, b))
    return tuple(roles)


def make_in_maps(x, target, W, b):
    x = np.ascontiguousarray(np.asarray(x), dtype=np.float32)
    tgt = np.asarray(target).astype(np.int64)
    Wm = np.asarray(W, dtype=np.float32).reshape(20 * OUT_CHANNELS, 3)
    Wsel = Wm.reshape(20, OUT_CHANNELS, 3)[tgt]  # (N, 3 out, 3 in)
    roles = _choose_roles(Wsel)

    xpad = np.zeros((N, 3, G32, FD + 1), dtype=np_bf16)
    xpad[:, :, :, :FD] = x.reshape(N, 3, G32, FD).astype(np_bf16)

    in_maps = []
    for core in range(NCORES):
        lo = core * NS
        xsc = np.ascontiguousarray(
            xpad[lo : lo + NS].transpose(1, 0, 2, 3).reshape(3, P, FD + 1)
        )
        aux = np.zeros((P, 272), dtype=np.float32)
        for j, (a, p, bb) in enumerate(roles):
            wp = Wsel[lo : lo + NS, j, p]          # (NS,)
            rra = Wsel[lo : lo + NS, j, a] / wp
            rrb = Wsel[lo : lo + NS, j, bb] / wp
            sg = np.sign(wp).astype(np.float32)
            aux[:, 2 * j] = np.repeat(rra, G32)
            aux[:, 2 * j + 1] = np.repeat(rrb, G32)
            aux[:NS, 6 + j] = 2.0 * OUT_SCALE * sg
            aux[:NS, 9 + j] = -OUT_SCALE * sg
        aux[0, 12] = 1.0
        aux[1, 13] = 1.0
        aux[:, 16 : 16 + P] = np.eye(P, dtype=np.float32)
        aux[:NS, 16 + P : 16 + 2 * P] = np.repeat(
            np.eye(NS, dtype=np.float32), G32, axis=1
        )
        in_maps.append({"xs": xsc, "aux": aux})
    return in_maps, roles


def run(x, target, W, b, trace=False, retries=2):
    in_maps, roles = make_in_maps(x, target, W, b)
    nc = get_nc(roles)
    last_err = None
    for attempt in range(retries + 1):
        try:
            res = run_bass_kernel_spmd(
                nc, in_maps, list(range(NCORES)), trace=trace
            )
            outs = []
            for r in res.results:
                o = np.asarray(r["out"]).astype(np.float32)
                o = o.reshape(3, NS, G32, FD).transpose(1, 0, 2, 3)
                outs.append(o.reshape(NS, OUT_CHANNELS, H, W_))
            return np.concatenate(outs, axis=0), res
        except Exception as e:  # device may need recovery; retry
            last_err = e
            if attempt < retries:
                time.sleep(20)
    raise last_err


def kernel(x, target, W, b):
    out, _ = run(x, target, W, b)
    return out
